# revision 1
# baseline (speedup 1.0000x reference)
"""APPNP-style GNN message passing on 8 Trainium2 NeuronCores.

Math (matches the PyG-default reference):
    h0 = (relu(x @ w1 + b1)) @ w2 + b2                       # MLP, [N, C]
    deg[v] = indegree(v) + 1 ; dinv = 1/sqrt(deg)
    repeat K times:
        h <- (1-a) * D^-1/2 (A + I) D^-1/2 h + a * h0
    out = log_softmax(h, axis=1)

Distribution (8 cores):
  * Nodes degree-sorted and dealt into t tiles of 128 destinations per
    core.  Flat table row id of node (c, k, p) is (c*128 + p)*t + k.
  * Every core keeps a replicated DRAM table of g = dinv * h.  Per hop a
    core gathers, for each destination tile, the source rows of all its
    incoming edges with `dma_gather` (int16 indices force grouping each
    tile's edges by 32768-row table window), then scatter-reduces them
    with PE matmuls against one-hot selection matrices built on DVE from
    an iota tile: psum[dest, f] += sum_pos S[pos, dest] * G[pos, f].
    The updated g shard is AllGathered into every core's table.
  * The gather/matmul schedule is compile-time and shared by all cores
    (chunk counts are the max over cores); padding positions gather
    window row 0 and carry dest id -1, so their one-hot column is zero.
"""

import sys

for _p in ("/opt/trn_rl_repo",):
    if _p not in sys.path:
        sys.path.insert(0, _p)

import numpy as np

import concourse.bacc as bacc
import concourse.mybir as mybir
import concourse.tile as tile

AF = mybir.ActivationFunctionType
ALU = mybir.AluOpType
DT = mybir.dt

N = 100000
E = 3200000
F_IN = 512
HID = 64
C = 64
K = 10
ALPHA = 0.1

R = 8            # cores
P = 128          # partitions
WINDOW = 32768   # int16 index reach of dma_gather
GROUP = 4        # dest tiles sharing one grid buffer / gather set


# --------------------------------------------------------------------------
# Host-side preprocessing
# --------------------------------------------------------------------------

def make_schedule(mcom, group=GROUP):
    """Common compile-time schedule from chunk counts mcom [t, nwin].

    Position layout: for each tile-group g, for each window q, for each
    tile k in g: a band of mcom[k, q]*128 positions.  One dma_gather per
    (g, q) covers that (contiguous) run.  Returns list of group dicts.
    """
    t, nwin = mcom.shape
    sched = []
    pos0 = 0
    for g0 in range(0, t, group):
        tiles = list(range(g0, min(g0 + group, t)))
        gathers = []
        tile_chunks = {k: [] for k in tiles}
        gslot = 0
        grp_pos0 = pos0
        for q in range(nwin):
            npos = int(mcom[tiles, q].sum()) * P
            if npos == 0:
                continue
            gathers.append((pos0, npos, gslot, q))
            for k in tiles:
                m = int(mcom[k, q])
                if m:
                    tile_chunks[k].append((gslot, m))
                    gslot += m
            pos0 += npos
        sched.append({
            "gathers": gathers,                  # (pos0, npos, slot0, q)
            "tiles": [(k, tile_chunks[k]) for k in tiles],
            "mg": gslot,
            "pos0": grp_pos0,
        })
    return sched, pos0                           # total positions


def preprocess(edge_index, n, r=R, p=P, group=GROUP):
    row = np.asarray(edge_index[0], dtype=np.int64)
    col = np.asarray(edge_index[1], dtype=np.int64)

    indeg = np.bincount(col, minlength=n)
    deg = indeg + 1                              # + self loop
    dinv = (1.0 / np.sqrt(deg.astype(np.float64))).astype(np.float32)

    block = r * p
    t = -(-n // block)
    nt = block * t
    shard = p * t
    nwin = -(-nt // WINDOW)

    order = np.argsort(-deg, kind="stable")
    sorted_nodes = np.concatenate([order, np.arange(n, nt)])
    q_of = np.empty(nt, dtype=np.int64)
    q_of[sorted_nodes] = np.arange(nt)

    k_of = q_of // block
    c_of = (q_of % block) // p
    p_of = q_of % p
    rowid = (c_of * p + p_of) * t + k_of         # node -> table row

    srcs = np.concatenate([row, np.arange(n, dtype=np.int64)])
    dsts = np.concatenate([col, np.arange(n, dtype=np.int64)])
    src_rid = rowid[srcs]
    win = src_rid // WINDOW

    keys = (c_of[dsts] * t + k_of[dsts]) * nwin + win
    eorder = np.argsort(keys, kind="stable")
    e_key = keys[eorder]
    e_src = src_rid[eorder]
    e_win = win[eorder]
    e_dst_p = p_of[dsts][eorder]

    counts = np.bincount(e_key, minlength=r * t * nwin).reshape(r, t, nwin)
    mcom = (-(-counts // p)).max(axis=0)                     # [t, nwin]
    # every tile needs >= 1 chunk so its PSUM accumulator is initialized
    mcom[:, 0] = np.maximum(mcom[:, 0], 1)

    sched, tot_pos = make_schedule(mcom, group)
    tot_slot = tot_pos // p

    # band start positions under the common schedule
    band_start = np.zeros((t, nwin), dtype=np.int64)
    pos0 = 0
    for g0 in range(0, t, group):
        tiles = range(g0, min(g0 + group, t))
        for q in range(nwin):
            for k in tiles:
                band_start[k, q] = pos0
                pos0 += int(mcom[k, q]) * p
    assert pos0 == tot_pos

    starts = np.concatenate([[0], np.cumsum(counts.reshape(-1))])
    erank = np.arange(e_key.shape[0]) - starts[e_key]
    e_c = e_key // (t * nwin)
    e_k = (e_key // nwin) % t
    e_pos = band_start[e_k, e_win] + erank

    idx16 = np.zeros((r, tot_pos), dtype=np.int16)
    dcol = np.full((r, p, tot_slot), -1.0, dtype=np.float32)
    idx16[e_c, e_pos] = (e_src - e_win * WINDOW).astype(np.int16)
    dcol[e_c, e_pos % p, e_pos // p] = e_dst_p.astype(np.float32)

    # wrap indices: j -> [j%16, j//16], replicated over the 8 Q7 groups
    w16 = idx16.reshape(r, tot_pos // 16, 16).transpose(0, 2, 1)
    idx16w = np.ascontiguousarray(np.tile(w16, (1, 8, 1)))   # [r, 128, tp/16]

    inv = np.empty(nt, dtype=np.int64)
    inv[rowid] = np.arange(nt)
    shard_nodes = inv.reshape(r, shard)

    dinv_pad = np.zeros(nt, dtype=np.float32)
    dinv_pad[:n] = dinv
    dinv_pk = dinv_pad[shard_nodes].reshape(r, p, t)

    return {
        "t": t, "nt": nt, "shard": shard, "nwin": nwin,
        "mcom": mcom, "sched": sched, "tot_pos": tot_pos,
        "tot_slot": tot_slot,
        "idx16w": idx16w, "dcol": np.ascontiguousarray(dcol),
        "rowid": rowid, "shard_nodes": shard_nodes,
        "dinv_pk": np.ascontiguousarray(dinv_pk),
    }


# --------------------------------------------------------------------------
# Bass program
# --------------------------------------------------------------------------

def build_program(t, nt, nwin, sched, tot_pos, tot_slot,
                  f_in=F_IN, k_hops=K, alpha=ALPHA, r=R,
                  no_gather=False, no_sgen=False, no_mm=False, no_ag=False,
                  nqueues=1):
    shard = P * t
    fc = f_in // P
    assert f_in % P == 0

    nc = bacc.Bacc("TRN2", target_bir_lowering=False, debug=False,
                   num_devices=r)

    xT = nc.dram_tensor("xT", [f_in, shard], DT.float32, kind="ExternalInput")
    w1r = nc.dram_tensor("w1r", [P, fc, HID], DT.float32, kind="ExternalInput")
    b1c = nc.dram_tensor("b1c", [HID, 1], DT.float32, kind="ExternalInput")
    w2m = nc.dram_tensor("w2m", [HID, C], DT.float32, kind="ExternalInput")
    b2r = nc.dram_tensor("b2r", [P, C], DT.float32, kind="ExternalInput")
    dinv_h = nc.dram_tensor("dinv", [P, t], DT.float32, kind="ExternalInput")
    dinv09_h = nc.dram_tensor("dinv09", [P, t], DT.float32,
                              kind="ExternalInput")
    idx_h = nc.dram_tensor("idx16", [P, tot_pos // 16], DT.int16,
                           kind="ExternalInput")
    dcol_h = nc.dram_tensor("dcol", [P, tot_slot], DT.float32,
                            kind="ExternalInput")
    iota_h = nc.dram_tensor("iota", [P, P], DT.float32, kind="ExternalInput")
    out_h = nc.dram_tensor("out", [P, t * C], DT.float32,
                           kind="ExternalOutput")

    groups = [list(range(r))]
    mg_max = max(g["mg"] for g in sched)

    with tile.TileContext(nc) as tc:
        with (
            tc.tile_pool(name="const", bufs=1) as cpool,
            tc.tile_pool(name="xin", bufs=3) as xpool,
            tc.tile_pool(name="mlp", bufs=3) as mpool,
            tc.tile_pool(name="grid", bufs=2) as gpool,
            tc.tile_pool(name="idxp", bufs=6) as ipool,
            tc.tile_pool(name="sel", bufs=4) as spool,
            tc.tile_pool(name="small", bufs=6) as apool,
            tc.tile_pool(name="cols", bufs=6) as colpool,
            tc.tile_pool(name="ps", bufs=2, space="PSUM") as pspool,
            tc.tile_pool(name="psb", bufs=3, space="PSUM") as psbpool,
            tc.tile_pool(name="dram", bufs=1, space="DRAM") as dpool,
        ):
            bounce = dpool.tile([P, t * C], DT.float32)
            table = dpool.tile([r * P, t * C], DT.float32)

            w1_s = cpool.tile([P, fc, HID], DT.float32)
            b1_s = cpool.tile([HID, 1], DT.float32)
            w2_s = cpool.tile([HID, C], DT.float32)
            b2_s = cpool.tile([P, C], DT.float32)
            dinv_s = cpool.tile([P, t], DT.float32)
            dinv09_s = cpool.tile([P, t], DT.float32)
            dcol_s = cpool.tile([P, tot_slot], DT.float32)
            iota_s = cpool.tile([P, P], DT.float32)
            h0s_buf = cpool.tile([P, t * C], DT.float32)   # 0.1 * h0
            g_buf = cpool.tile([P, t * C], DT.float32)     # dinv * h

            nc.sync.dma_start(out=w1_s[:], in_=w1r.ap())
            nc.sync.dma_start(out=b1_s[:], in_=b1c.ap())
            nc.sync.dma_start(out=w2_s[:], in_=w2m.ap())
            nc.sync.dma_start(out=b2_s[:], in_=b2r.ap())
            nc.sync.dma_start(out=dinv_s[:], in_=dinv_h.ap())
            nc.sync.dma_start(out=dinv09_s[:], in_=dinv09_h.ap())
            nc.sync.dma_start(out=dcol_s[:], in_=dcol_h.ap())
            nc.sync.dma_start(out=iota_s[:], in_=iota_h.ap())

            xT_r = xT.ap().rearrange("(c p) n -> p c n", p=P)
            table_flat = table[:].rearrange("a (k f) -> (a k) f", f=C)

            # ---------------- MLP ----------------------------------------
            for kt in range(t):
                ksl = slice(kt * C, (kt + 1) * C)
                xt = xpool.tile([P, fc, P], DT.float32, tag="xt")
                nc.sync.dma_start(out=xt[:], in_=xT_r[:, :, kt * P:(kt + 1) * P])
                ps1 = pspool.tile([HID, P], DT.float32, tag="ps1")
                for ci in range(fc):
                    nc.tensor.matmul(ps1[:], lhsT=w1_s[:, ci, :],
                                     rhs=xt[:, ci, :],
                                     start=(ci == 0), stop=(ci == fc - 1))
                h1 = mpool.tile([HID, P], DT.float32, tag="h1")
                nc.scalar.activation(h1[:], ps1[:], AF.Relu, bias=b1_s[:, 0:1])
                ps3 = pspool.tile([P, C], DT.float32, tag="ps3")
                nc.tensor.matmul(ps3[:], lhsT=h1[:], rhs=w2_s[:],
                                 start=True, stop=True)
                hb = mpool.tile([P, C], DT.float32, tag="hb")
                nc.vector.tensor_add(out=hb[:], in0=ps3[:], in1=b2_s[:])
                nc.scalar.mul(h0s_buf[:, ksl], hb[:], alpha)
                nc.vector.tensor_scalar_mul(g_buf[:, ksl], hb[:],
                                            dinv_s[:, kt:kt + 1])

            nc.sync.dma_start(out=bounce[:], in_=g_buf[:])
            nc.gpsimd.collective_compute(
                "AllGather", ALU.bypass, replica_groups=groups,
                ins=[bounce[:].opt()], outs=[table[:].opt()])

            # ---------------- K propagation hops -------------------------
            gq = 0
            for hop in range(k_hops):
                last = hop == k_hops - 1
                for grp in sched:
                    mg = grp["mg"]
                    gbase = grp["pos0"] // P          # global slot base
                    grid = gpool.tile([P, mg_max, C], DT.float32, tag="grid")
                    for (gpos0, npos, slot0, q) in grp["gathers"]:
                        if no_gather:
                            continue
                        ncols = npos // 16
                        it = ipool.tile([P, ncols], DT.int16, tag="idx")
                        nc.sync.dma_start(
                            out=it[:],
                            in_=idx_h.ap()[:, gpos0 // 16:gpos0 // 16 + ncols])
                        lo = q * WINDOW
                        hi = min(lo + WINDOW, nt)
                        nc.gpsimd.dma_gather(
                            out_ap=grid[:, slot0:slot0 + npos // P, :],
                            in_ap=table_flat[lo:hi, :],
                            idxs_ap=it[:],
                            num_idxs=npos,
                            num_idxs_reg=npos,
                            elem_size=C,
                            single_packet=False,
                            queue_num=gq % nqueues,
                        )
                        gq += 1
                    for (kt, chunks) in grp["tiles"]:
                        ksl = slice(kt * C, (kt + 1) * C)
                        psA = psbpool.tile([P, C], DT.float32, tag="agg")
                        nchunks = sum(m for _, m in chunks)
                        done = 0
                        for (slot0, m) in chunks:
                            for s in range(slot0, slot0 + m):
                                if no_sgen:
                                    sel = iota_s
                                else:
                                    sel = spool.tile([P, P], DT.float32,
                                                     tag="S")
                                    nc.vector.tensor_scalar(
                                        out=sel[:], in0=iota_s[:],
                                        scalar1=dcol_s[:,
                                                       gbase + s:gbase + s + 1],
                                        scalar2=None, op0=ALU.is_equal)
                                if not no_mm:
                                    nc.tensor.matmul(
                                        psA[:], lhsT=sel[:],
                                        rhs=grid[:, s, :],
                                        start=(done == 0),
                                        stop=(done == nchunks - 1))
                                done += 1
                        if no_mm:
                            nc.vector.tensor_copy(psA[:], grid[:, 0, :])
                        tmp = apool.tile([P, C], DT.float32, tag="tmp")
                        nc.scalar.activation(tmp[:], psA[:], AF.Identity,
                                             scale=dinv09_s[:, kt:kt + 1])
                        hn = apool.tile([P, C], DT.float32, tag="hn")
                        nc.vector.tensor_add(out=hn[:], in0=tmp[:],
                                             in1=h0s_buf[:, ksl])
                        if not last:
                            nc.vector.tensor_scalar_mul(
                                g_buf[:, ksl], hn[:], dinv_s[:, kt:kt + 1])
                        else:
                            mx = colpool.tile([P, 1], DT.float32, tag="mx")
                            nc.vector.reduce_max(mx[:], hn[:],
                                                 axis=mybir.AxisListType.X,
                                                 negate=True)       # -max
                            ex = apool.tile([P, C], DT.float32, tag="ex")
                            ssum = colpool.tile([P, 1], DT.float32, tag="ssum")
                            nc.scalar.activation(ex[:], hn[:], AF.Exp,
                                                 bias=mx[:, 0:1],
                                                 accum_out=ssum[:, 0:1])
                            lg = colpool.tile([P, 1], DT.float32, tag="lg")
                            nc.scalar.activation(lg[:], ssum[:], AF.Ln)
                            mpl = colpool.tile([P, 1], DT.float32, tag="mpl")
                            nc.vector.tensor_tensor(out=mpl[:], in0=lg[:],
                                                    in1=mx[:],
                                                    op=ALU.subtract)
                            nc.vector.tensor_scalar(
                                out=g_buf[:, ksl], in0=hn[:],
                                scalar1=mpl[:, 0:1], scalar2=None,
                                op0=ALU.subtract)
                if not last and not no_ag:
                    nc.sync.dma_start(out=bounce[:], in_=g_buf[:])
                    nc.gpsimd.collective_compute(
                        "AllGather", ALU.bypass, replica_groups=groups,
                        ins=[bounce[:].opt()], outs=[table[:].opt()])

            nc.sync.dma_start(out=out_h.ap(), in_=g_buf[:])

    nc.compile()
    return nc


# --------------------------------------------------------------------------
# in_maps assembly
# --------------------------------------------------------------------------

def make_in_maps(x, w1, b1, w2, b2, pre, f_in=F_IN, r=R):
    n = x.shape[0]
    t, nt = pre["t"], pre["nt"]
    fc = f_in // P

    xp = np.zeros((nt, f_in), dtype=np.float32)
    xp[:n] = np.asarray(x, dtype=np.float32)
    w1r = np.ascontiguousarray(
        np.asarray(w1, np.float32).reshape(fc, P, HID).transpose(1, 0, 2))
    b1c = np.ascontiguousarray(np.asarray(b1, np.float32).reshape(HID, 1))
    w2m = np.ascontiguousarray(np.asarray(w2, np.float32))
    b2r = np.ascontiguousarray(
        np.tile(np.asarray(b2, np.float32).reshape(1, C), (P, 1)))
    iota = np.ascontiguousarray(
        np.tile(np.arange(P, dtype=np.float32).reshape(1, P), (P, 1)))

    in_maps = []
    for c in range(r):
        nodes = pre["shard_nodes"][c].reshape(P, t).T.reshape(-1)  # k-major
        xT_c = np.ascontiguousarray(xp[nodes].T)
        dpk = pre["dinv_pk"][c]
        in_maps.append({
            "xT": xT_c,
            "w1r": w1r, "b1c": b1c, "w2m": w2m, "b2r": b2r,
            "dinv": np.ascontiguousarray(dpk),
            "dinv09": np.ascontiguousarray(0.9 * dpk),
            "idx16": pre["idx16w"][c],
            "dcol": pre["dcol"][c],
            "iota": iota,
        })
    return in_maps


_CACHE = {}


def kernel(x, edge_index, w1, b1, w2, b2):
    from concourse.bass_utils import run_bass_kernel_spmd

    x = np.asarray(x)
    n = x.shape[0]
    pre = preprocess(np.asarray(edge_index), n)
    key = (pre["t"], pre["tot_pos"], tuple(pre["mcom"].reshape(-1)))
    if key not in _CACHE:
        _CACHE[key] = build_program(pre["t"], pre["nt"], pre["nwin"],
                                    pre["sched"], pre["tot_pos"],
                                    pre["tot_slot"])
    nc = _CACHE[key]

    in_maps = make_in_maps(x, w1, b1, w2, b2, pre)
    res = run_bass_kernel_spmd(nc, in_maps, core_ids=list(range(R)))
    outs = np.stack([res.results[c]["out"] for c in range(R)])
    flat = outs.reshape(R * pre["shard"], C)
    return np.ascontiguousarray(flat[pre["rowid"][:n]]).astype(np.float32)

